# revision 28
# baseline (speedup 1.0000x reference)
"""Trainium2 Bass kernel: per-head attention + residual + LayerNorm.

Problem shape: x [4, 2048, 1024], 16 heads of dk=64, causal softmax attention
with per-head Q/K/V linear projections, residual add, LayerNorm(D).

Sharding (8 cores): head-parallel. Core i owns heads (2i, 2i+1) = feature
columns 128*i : 128*(i+1). Only cross-core traffic: per-(batch,unit) AllReduce
of LayerNorm partial sums.

v2 design (vs the 202us baseline, whose bottleneck was the ACT engine at 77%
busy running exp over the causal score area):
- Softmax-invariance scores: sp[t,s] = x_t^T (Wk Wq^T/sqrt(dk)) x_s + u.x_t
  (query-only bias terms cancel in softmax). The per-key bias u.x_t is
  computed on the HOST and folded into the exp instruction's per-partition
  bias operand - no bias-row augmentation, no on-device bias adds.
- zz = block-diag(A_h0^T, A_h1^T) @ xx projects BOTH heads in one matmul
  stream ([128,S]); per-head scores contract over a 64-partition slice.
- exp is split across the ACT engine (true Exp) and DVE (Schraudolph:
  bf16 = bitcast(int16(184.665*s + bias)), a single tensor_scalar with
  ~1.8% sigma error that mostly cancels in softmax). A build-time greedy
  balancer also assigns the zz/v PSUM->SBUF copies to ACT (AF.Copy, same
  act table) or DVE. GPSIMD cannot touch PSUM, so Pool instead absorbs the
  SBUF-only LayerNorm/stats elementwise work.
- PV reoriented to P^T V: out[s-chunk 128, 65] with lhsT = P chunk; free
  size 65 per matmul (vs 512) halves PE PV time, writes y directly (no PE
  transposes, no O^T drain copies), and the rhs ones-column lands the
  softmax denominator in chunk col 64. V bias is pre-added into xs on host.
- Query-units of (896, 896, 256) cols keep the PV chunk tile (nch x 65 fp32
  <= 455) inside one PSUM bank; processed [u1, u2, u0] so the last
  (batch,unit) stats AllReduce covers the smallest unit (short tail).
- LayerNorm output (emit_ln) is emitted ~one pair after its AllReduce
  fires, spreading Pool/out-DMA work over compute instead of a serial tail.

Self-contained: hardcodes all shapes; no sibling imports.
"""

import os
import numpy as np
import ml_dtypes

import concourse.bass as bass
import concourse.bacc as bacc
import concourse.mybir as mybir
from concourse.tile import TileContext
from concourse.bass_utils import run_bass_kernel_spmd

B, S, D, H = 4, 2048, 1024, 16
NCORES = 8
HPC = H // NCORES          # heads per core = 2
DK = D // H                # 64
DC = HPC * DK              # 128 feature cols per core
NT = S // 128              # 16 row tiles of 128
EPS = 1e-5
MASKNEG = -40.0
EPI_LAG = 1
PV_LAG = int(os.environ.get("K_PV_LAG", "3"))
PBUFS = 18
BF = mybir.dt.bfloat16
F32 = mybir.dt.float32
I16 = mybir.dt.int16
BF_NP = ml_dtypes.bfloat16
RG = [list(range(NCORES))]
A = mybir.AluOpType
AF = mybir.ActivationFunctionType

# Schraudolph exp constants (bf16 = bitcast(int16(SCH_A * v + SCH_B)))
SCH_A = 184.6650292  # 2^7 / ln 2
SCH_B = 16256.0 - 7.32  # 127 * 2^7, centered (hw rounds to nearest)

# Query units (hs, he, nch): 512-wide so the score tile is one PSUM bank
# (4-deep sp ring hides exp latency). Order keeps a long unit first (proj
# hooks) and last (tail LN drain room).
UNITS = [(0, 512, 4), (512, 1024, 4), (1024, 1536, 4), (1536, 2048, 4)]
PORDER = [1, 2, 0, 3]
NU = len(UNITS)


def _units_for(S_):
    if S_ == 2048:
        return UNITS, PORDER
    nch = S_ // 128
    assert nch <= 7
    return [(0, S_, nch)], [0]

# greedy ACT/DVE balance cost model (ns): per-instr, per-col
ACT_COST = (190.0, 0.833)
DVE_COST = (127.0, 1.042)
# recurring per-pair engine loads not part of the flexible item list
DVE_PAIR_FIXED = float(os.environ.get("K_DVE_PAIR", "3700"))
ACT_PAIR_FIXED = float(os.environ.get("K_ACT_PAIR", "1250"))

LAST_RESULTS = None  # BassKernelResults of the last run (for test harness)


def _exp_schedule(B_=B, S_=S):
    """Greedy ACT/DVE balance over exp tiles and psum->sbuf copies in
    emission order. Returns {key: 'act'|'dve'}."""
    units, porder = _units_for(S_)
    NT_ = S_ // 128
    load = {"act": 0.0, "dve": 0.0}
    sched = {}

    def assign(key, n):
        cost_a = ACT_COST[0] + ACT_COST[1] * n
        cost_d = DVE_COST[0] + DVE_COST[1] * n
        if load["act"] + cost_a <= load["dve"] + cost_d:
            sched[key] = "act"
            load["act"] += cost_a
        else:
            sched[key] = "dve"
            load["dve"] += cost_d

    for pair in range(B_ * HPC):
        b, hh = divmod(pair, HPC)
        load["dve"] += DVE_PAIR_FIXED
        load["act"] += ACT_PAIR_FIXED
        if hh == 0:
            for c in range(S_ // 512):
                assign(("z", b, c), 512)
        for g in range((NT_ + 7) // 8):
            assign(("v", pair, g), 64 * min(8, NT_ - 8 * g))
        for u in porder:
            hs, he, nch = units[u]
            w = he - hs
            npre = hs // 128  # pre-diagonal j's, merged in pairs
            for jp in range(npre // 2):
                assign((pair, u, "m", jp), 2 * w)
            for j in range(npre, he // 128):
                lo = 128 * j - hs
                assign((pair, u, j), w - lo)
            if hh == 1:
                for i in range(nch):
                    # sum-of-squares: ACT Square+accum vs DVE stt
                    cost_a = 190.0 + 187.0 + 128 * ACT_COST[1]
                    cost_d = DVE_COST[0] + 128 * DVE_COST[1]
                    if load["act"] + cost_a <= load["dve"] + cost_d:
                        sched[("sq", b, u, i)] = "act"
                        load["act"] += cost_a
                    else:
                        sched[("sq", b, u, i)] = "dve"
                        load["dve"] += cost_d
    return sched


def _build_graph(apply_affine: bool, B_: int = B, S_: int = S, rg=None, fake_ar: bool = False) -> bass.Bass:
    nc = bacc.Bacc()
    NT_ = S_ // 128
    if rg is None:
        rg = RG
    sched = _exp_schedule(B_, S_)
    units_, porder_ = _units_for(S_)
    NU_ = len(units_)

    xx = nc.declare_dram_parameter("xx", [B_, DC, S_], BF, isOutput=False)
    xs = nc.declare_dram_parameter("xs", [B_, S_, DC], F32, isOutput=False)
    wpack = nc.declare_dram_parameter("wpack", [DC, DC + DK], BF, isOutput=False)
    # per-key bias exp(u.x_t) is folded multiplicatively into V: xv is x
    # pre-scaled by e^bb per head-half (V-projection input), ebb fills the
    # denominator column of v3.
    xv = nc.declare_dram_parameter("xv", [B_, DC, S_], BF, isOutput=False)
    ebb = nc.declare_dram_parameter("ebb", [B_, HPC, 128, NT_], F32, isOutput=False)
    if apply_affine:
        gam = nc.declare_dram_parameter("gam", [128, DC], F32, isOutput=False)
        bet = nc.declare_dram_parameter("bet", [128, DC], F32, isOutput=False)
    out = nc.declare_dram_parameter("out", [B_, S_, DC], F32, isOutput=True)
    dbg_y = None
    if os.environ.get("K_DBG_Y"):
        dbg_y = nc.declare_dram_parameter("dbg_y", [B_, 128, S_], F32, isOutput=True)
    dbg_p = None
    if os.environ.get("K_DBG_P"):
        dbg_p = nc.declare_dram_parameter("dbg_p", [NT_, 128, 512], F32, isOutput=True)
        dbg_ch = nc.declare_dram_parameter("dbg_ch", [128, 260], F32, isOutput=True)

    # constants baked into the NEFF: [idn128 | upper-triangle MASKNEG] bf16
    trineg_np = np.where(
        np.arange(128)[:, None] > np.arange(128)[None, :], MASKNEG, 0.0
    ).astype(np.float32)
    imask_h = nc.inline_tensor(
        np.concatenate([np.eye(128, dtype=np.float32), trineg_np], axis=1).astype(BF_NP),
        name="imask",
    )

    # collective bounce buffers: LayerNorm stats per (batch, unit):
    # [2(sum,sumsq), 128 rows, chunks]
    stats_in = nc.dram_tensor("stats_in", [B_, NU_, 2, 128, 7], F32)
    stats_out = nc.dram_tensor("stats_out", [B_, NU_, 2, 128, 7], F32, addr_space="Shared")

    NP = B_ * HPC

    def copy_ps(key, dst, src):
        if sched[key] == "act":
            nc.scalar.activation(dst, src, AF.Copy)
        else:
            nc.vector.tensor_copy(dst, src)

    with TileContext(nc) as tc:
        with (
            tc.tile_pool(name="consts", bufs=1) as cpool,
            tc.tile_pool(name="sb", bufs=2) as sb,
            tc.tile_pool(name="ps", bufs=1, space="PSUM") as ps,
        ):
            # ---- constants (first-needed first) ----
            wp_t = cpool.tile([DC, DC + DK], BF, tag="wp")
            nc.sync.dma_start(out=wp_t[:], in_=wpack[:, :])
            wz_t = wp_t[:][:, 0:DC]
            wv_t = wp_t[:][:, DC : DC + DK]
            imaskq_t = cpool.tile([128, 256], BF, tag="imask")
            nc.gpsimd.dma_start(out=imaskq_t[:], in_=imask_h[:, :])
            idn128_t = imaskq_t[:][:, 0:128]
            maskt_t = imaskq_t[:][:, 128:256]
            eps_t = cpool.tile([128, 1], F32, tag="eps")
            nc.vector.memset(eps_t[:], EPS)
            if apply_affine:
                gam_t = cpool.tile([128, DC], F32, tag="gam")
                nc.gpsimd.dma_start(out=gam_t[:], in_=gam[:, :])
                bet_t = cpool.tile([128, DC], F32, tag="bet")
                nc.gpsimd.dma_start(out=bet_t[:], in_=bet[:, :])

            y_tiles = {}
            bstate = {}
            pstate = {}
            pending_ln = []

            def _emit_stats(b, u, y_b, accs):
                hs, he, nch = units_[u]
                t0 = hs // 128
                pk = sb.tile([128, 14], F32, tag="pk", bufs=3)
                nc.gpsimd.tensor_add(
                    pk[:, 0:nch], accs[0][:, t0 : t0 + nch], accs[1][:, t0 : t0 + nch]
                )
                for i in range(t0, t0 + nch):
                    scr = sb.tile([128, 128], F32, tag="scr")
                    if sched[("sq", b, u, i - t0)] == "act":
                        nc.scalar.activation(
                            scr[:],
                            y_b[:, 128 * i : 128 * i + 128],
                            AF.Square,
                            accum_out=pk[:, nch + i - t0 : nch + 1 + i - t0],
                        )
                    else:
                        nc.vector.scalar_tensor_tensor(
                            scr[:],
                            y_b[:, 128 * i : 128 * i + 128],
                            1.0,
                            y_b[:, 128 * i : 128 * i + 128],
                            A.mult,
                            A.mult,
                            accum_out=pk[:, nch + i - t0 : nch + 1 + i - t0],
                        )
                nc.sync.dma_start(
                    out=stats_in[b, u, :, :, 0:nch].rearrange("c p t -> p c t"),
                    in_=pk[:, 0 : 2 * nch].rearrange("p (c t) -> p c t", t=nch),
                )
                if dbg_y is not None:
                    nc.sync.dma_start(
                        out=dbg_y[b, :, hs:he], in_=y_b[:, hs:he]
                    )
                if fake_ar:
                    nc.sync.dma_start(
                        out=stats_out[b, u, :, :, 0:nch], in_=stats_in[b, u, :, :, 0:nch]
                    )
                else:
                    nc.gpsimd.collective_compute(
                        "AllReduce",
                        A.add,
                        replica_groups=rg,
                        ins=[stats_in[b, u].opt()],
                        outs=[stats_out[b, u].opt()],
                    )
                pending_ln.append(lambda b=b, u=u: emit_ln(b, u, y_b))

            def emit_ln(b, u, y_b):
                hs, he, nch = units_[u]
                t0 = hs // 128
                red = sb.tile([128, 14], F32, tag="red", bufs=3)
                nc.sync.dma_start(
                    out=red[:, 0 : 2 * nch].rearrange("p (c t) -> p c t", t=nch),
                    in_=stats_out[b, u, :, :, 0:nch].rearrange("c p t -> p c t"),
                )
                mean = sb.tile([128, 7], F32, tag="mean", bufs=3)
                nc.gpsimd.tensor_scalar(mean[:, 0:nch], red[:, 0:nch], 1.0 / D, None, A.mult)
                msq = sb.tile([128, 7], F32, tag="msq", bufs=3)
                nc.gpsimd.tensor_mul(msq[:, 0:nch], mean[:, 0:nch], mean[:, 0:nch])
                var = sb.tile([128, 7], F32, tag="var", bufs=3)
                nc.gpsimd.tensor_scalar(
                    var[:, 0:nch], red[:, nch : 2 * nch], 1.0 / D, None, A.mult
                )
                nc.gpsimd.tensor_tensor(
                    var[:, 0:nch], var[:, 0:nch], msq[:, 0:nch], A.subtract
                )
                lnv = sb.tile([128, 7], F32, tag="lnv", bufs=3)
                nc.scalar.activation(lnv[:, 0:nch], var[:, 0:nch], AF.Ln, bias=eps_t[:])
                rstd = sb.tile([128, 7], F32, tag="rstd", bufs=3)
                nc.scalar.activation(rstd[:, 0:nch], lnv[:, 0:nch], AF.Exp, scale=-0.5)
                ostb = sb.tile([128, 128 * 7], F32, tag="ost", bufs=2)
                for k in range(nch):
                    i = t0 + k
                    nc.gpsimd.tensor_scalar(
                        ostb[:, 128 * k : 128 * k + 128],
                        y_b[:, 128 * i : 128 * i + 128],
                        mean[:, k : k + 1],
                        rstd[:, k : k + 1],
                        A.subtract,
                        A.mult,
                    )
                    if apply_affine:
                        nc.gpsimd.tensor_mul(
                            ostb[:, 128 * k : 128 * k + 128],
                            ostb[:, 128 * k : 128 * k + 128],
                            gam_t[:],
                        )
                        nc.gpsimd.tensor_add(
                            ostb[:, 128 * k : 128 * k + 128],
                            ostb[:, 128 * k : 128 * k + 128],
                            bet_t[:],
                        )
                nc.sync.dma_start(
                    out=out[b, 128 * t0 : 128 * (t0 + nch), :].rearrange(
                        "(i p) d -> p i d", p=128
                    ),
                    in_=ostb[:, 0 : 128 * nch].rearrange("p (i d) -> p i d", d=128),
                )

            def emit_proj(pair):
                b, hh = divmod(pair, HPC)
                if hh == 0:
                    xx_b = sb.tile([128, S_], BF, tag="xx", name=f"xx{b}", bufs=2)
                    nc.sync.dma_start(out=xx_b[:, 0 : S_ // 2], in_=xx[b, :, 0 : S_ // 2])
                    nc.sync.dma_start(out=xx_b[:, S_ // 2 : S_], in_=xx[b, :, S_ // 2 : S_])
                    bb_t = sb.tile([128, HPC * NT_], F32, tag="bb", name=f"bb{b}", bufs=2)
                    nc.sync.dma_start(
                        out=bb_t[:].rearrange("p (h j) -> p h j", h=HPC),
                        in_=ebb[b].rearrange("h p j -> p h j"),
                    )
                    xv_b = sb.tile([128, S_], BF, tag="xv", name=f"xv{b}", bufs=2)
                    nc.sync.dma_start(out=xv_b[:, 0 : S_ // 2], in_=xv[b, :, 0 : S_ // 2])
                    nc.sync.dma_start(out=xv_b[:, S_ // 2 : S_], in_=xv[b, :, S_ // 2 : S_])
                    # zz projection: both heads at once via block-diag wz
                    zz_b = sb.tile([128, S_], BF, tag="zz", name=f"zz{b}", bufs=2)
                    for c in range(S_ // 512):
                        zp = ps.tile([128, 512], F32, tag="op", bufs=2, name=f"zp{b}_{c}")
                        nc.tensor.matmul(
                            zp[:],
                            lhsT=wz_t,
                            rhs=xx_b[:, 512 * c : 512 * c + 512],
                            start=True,
                            stop=True,
                        )
                        copy_ps(("z", b, c), zz_b[:, 512 * c : 512 * c + 512], zp[:])
                    xs_b = sb.tile([128, S_], F32, tag="xs", name=f"xs{b}")
                    nc.sync.dma_start(
                        out=xs_b[:].rearrange("p (i d) -> p i d", d=128),
                        in_=xs[b].rearrange("(i p) d -> p i d", p=128),
                    )
                    y_b = sb.tile([128, S_], F32, tag="y", name=f"y{b}", bufs=3)
                    y_tiles[b] = y_b
                    bstate[b] = (xx_b, zz_b, xs_b, y_b, bb_t, {}, xv_b)
                xx_b, zz_b, xs_b, y_b, bb_t, accs, xv_b = bstate[b]
                acc_h = sb.tile([128, NT_], F32, tag=f"acc{hh}", name=f"acc{pair}", bufs=2)
                accs[hh] = acc_h
                pstate[pair] = [None, acc_h]

            def emit_proj_v(pair):
                b, hh = divmod(pair, HPC)
                xx_b, zz_b, xs_b, y_b, bb_t, accs, xv_b = bstate[b]
                # V projection (from e^bb-scaled x): v[t, 65j:65j+64],
                # denominator column 64 carries e^bb.
                v_t = sb.tile([128, NT_ * 65], BF, tag="v", name=f"v{pair}", bufs=3)
                v3 = v_t[:].rearrange("p (t w) -> p t w", w=65)
                nc.gpsimd.tensor_copy(
                    v3[:, :, 64], bb_t[:, NT_ * hh : NT_ * hh + NT_]
                )
                for g in range((NT_ + 7) // 8):
                    gn = min(8, NT_ - 8 * g)
                    vp = ps.tile([128, 512], F32, tag="op", bufs=2, name=f"vp{pair}_{g}")
                    for uu in range(gn):
                        j = 8 * g + uu
                        nc.tensor.matmul(
                            vp[:, DK * uu : DK * uu + DK],
                            lhsT=xv_b[:][DK * hh : DK * hh + DK, 128 * j : 128 * j + 128],
                            rhs=wv_t[DK * hh : DK * hh + DK, :],
                            start=True,
                            stop=True,
                        )
                    copy_ps(
                        ("v", pair, g),
                        v3[:, 8 * g : 8 * g + gn, 0:64],
                        vp[:, 0 : DK * gn].rearrange("q (t w) -> q t w", w=DK),
                    )
                pstate[pair][0] = v3

            def emit_junit(pair, u, hooks=None):
                """Score/exp/PV loop for one query unit; returns the deferred
                normalize epilogue closure."""
                b, hh = divmod(pair, HPC)
                hs, he, nch = units_[u]
                w = he - hs
                xx_b, zz_b, xs_b, y_b, bb_t, accs, xv_b = bstate[b]
                hooks = dict(hooks or {})
                xh = xx_b[:][DK * hh : DK * hh + DK, :]
                zh = zz_b[:][DK * hh : DK * hh + DK, :]
                ch_t = ps.tile([128, 260], F32, tag="ch", bufs=2)
                npre = hs // 128  # pre-diagonal key tiles (even count)
                pviews = []
                bursts = []
                spt = None
                for j in range(he // 128):
                    s0 = 128 * j
                    rel = s0 - hs
                    lo = max(0, rel)
                    if j % 2 == 0:
                        spt = ps.tile([128, 1024], F32, tag="sp", bufs=2)
                    half = 512 * (j % 2)
                    sp = spt
                    # score matmuls; each span stays in one psum bank (the
                    # 512-col halves are bank-aligned)
                    def score_span(cs, ce, half=half):
                        while cs < ce:
                            sl = min(512 * (cs // 512) + 512, ce) - cs
                            nc.tensor.matmul(
                                sp[:, half + cs : half + cs + sl],
                                lhsT=xh[:, s0 : s0 + 128],
                                rhs=zh[:, hs + cs : hs + cs + sl],
                                start=True,
                                stop=True,
                                skip_group_check=True,
                            )
                            cs += sl
                    if rel < 0:
                        score_span(0, w)
                    else:
                        nc.tensor.matmul(
                            sp[:, half + rel : half + rel + 128],
                            lhsT=idn128_t,
                            rhs=maskt_t,
                            start=True,
                            stop=False,
                            skip_group_check=True,
                        )
                        nc.tensor.matmul(
                            sp[:, half + rel : half + rel + 128],
                            lhsT=xh[:, s0 : s0 + 128],
                            rhs=zh[:, s0 : s0 + 128],
                            start=False,
                            stop=True,
                            skip_group_check=True,
                        )
                        score_span(rel + 128, w)
                    if j < npre:
                        # biasless exp (e^bb folded into V): merged over the
                        # full [128,1024] sp tile once both halves are scored
                        if j % 2 == 1:
                            key = (pair, u, "m", j // 2)
                            if sched[key] == "act":
                                pm = sb.tile([128, 1024], BF, tag="pam", bufs=8)
                                nc.scalar.activation(pm[:], sp[:], AF.Exp)
                                pv = pm[:]
                            else:
                                pm = sb.tile([128, 1024], I16, tag="pim", bufs=8)
                                nc.vector.tensor_scalar(
                                    pm[:], sp[:], SCH_A, SCH_B, A.mult, A.add
                                )
                                pv = pm[:].bitcast(BF)
                            pviews.append((pv, 0))
                            pviews.append((pv, 512))
                    else:
                        if sched[(pair, u, j)] == "act":
                            p = sb.tile([128, 512], BF, tag="pa", bufs=PBUFS)
                            nc.scalar.activation(
                                p[:, lo:w], sp[:, half + lo : half + w], AF.Exp
                            )
                            pv = p[:]
                        else:
                            p = sb.tile([128, 512], I16, tag="pi", bufs=PBUFS)
                            nc.vector.tensor_scalar(
                                p[:, lo:w], sp[:, half + lo : half + w],
                                SCH_A, SCH_B, A.mult, A.add,
                            )
                            pv = p[:].bitcast(BF)
                        pviews.append((pv, 0))

                    if dbg_p is not None and pair == 0 and j >= npre:
                        pf = sb.tile([128, 512], F32, tag="pf", bufs=2)
                        nc.vector.tensor_copy(pf[:, lo:w], pv[:, lo:w])
                        nc.sync.dma_start(out=dbg_p[j, :, lo:w], in_=pf[:, lo:w])

                    # PSUM allows one open accumulation group per bank, so a
                    # chunk's PV contributions are emitted as one contiguous
                    # open->close burst once its diagonal P tile exists;
                    # deferred by one j so PE isn't gated on exp latency.
                    def _burst(c=j - hs // 128):
                        v3 = pstate[pair][0]
                        for jj in range(hs // 128 + c + 1):
                            pv, base = pviews[jj]
                            nc.tensor.matmul(
                                ch_t[:, 65 * c : 65 * c + 65],
                                lhsT=pv[:, base + 128 * c : base + 128 * c + 128],
                                rhs=v3[:, jj, :],
                                start=(jj == 0),
                                stop=(jj == hs // 128 + c),
                                skip_group_check=True,
                            )

                    if rel >= 0:
                        bursts.append(_burst)
                    if len(bursts) > PV_LAG:
                        bursts.pop(0)()
                    if j in hooks:
                        hooks.pop(j)()
                while bursts:
                    bursts.pop(0)()
                for hk in hooks.values():  # unit shorter than hook points
                    hk()

                def _epilogue():
                    t0 = hs // 128
                    if dbg_p is not None and pair == 0:
                        chf = sb.tile([128, 260], F32, tag="chf", bufs=2)
                        nc.vector.tensor_copy(chf[:, 0 : 65 * nch], ch_t[:, 0 : 65 * nch])
                        nc.sync.dma_start(out=dbg_ch[:, 0 : 65 * nch], in_=chf[:, 0 : 65 * nch])
                    acc_h = pstate[pair][1]
                    r7 = sb.tile([128, 7], F32, tag="r7", bufs=3)
                    nc.vector.reciprocal(
                        r7[:, 0:nch],
                        ch_t[:].rearrange("p (c w) -> p c w", w=65)[:, 0:nch, 64:65],
                    )
                    for c in range(nch):
                        i = t0 + c
                        nc.vector.scalar_tensor_tensor(
                            y_b[:, 128 * i + DK * hh : 128 * i + DK * hh + DK],
                            ch_t[:, 65 * c : 65 * c + 64],
                            r7[:, c : c + 1],
                            xs_b[:, 128 * i + DK * hh : 128 * i + DK * hh + DK],
                            A.mult,
                            A.add,
                            accum_out=acc_h[:, i : i + 1],
                        )
                    if hh == HPC - 1:
                        _emit_stats(b, u, y_b, accs)

                return _epilogue

            emit_proj(0)
            emit_proj_v(0)
            pending = []

            def pop_epi(lag=EPI_LAG):
                if len(pending) > lag:
                    pending.pop(0)()

            for pair in range(NP):
                def drain_ln(pair=pair):
                    if pending_ln and (len(pending_ln) >= 2 or pair >= NP - 2):
                        pending_ln.pop(0)()
                last = pair == NP - 1
                for k, u in enumerate(porder_):
                    hooks = {2: pop_epi, 8: drain_ln, 12: drain_ln}
                    if last:
                        hooks[5] = (lambda: pop_epi(0))
                        hooks[10] = drain_ln
                        hooks[14] = drain_ln
                    if k == 0 and pair + 1 < NP:
                        hooks[3] = (lambda pr=pair + 1: emit_proj(pr))
                        hooks[6] = (lambda pr=pair + 1: emit_proj_v(pr))
                    epi = emit_junit(pair, u, hooks=hooks)
                    pending.append(epi)
            for e in pending:
                e()
            while pending_ln:
                pending_ln.pop(0)()

    # Restrict Exp/Ln/Copy to the shared natural_log_exp_and_others table set
    # so the whole kernel uses one ACT table load.
    import concourse.bacc as _bacc_mod

    _orig_tables = _bacc_mod.get_activation_tables

    def _filtered_tables(arch):
        outm = {}
        for name, fns in _orig_tables(arch).items():
            if name != "natural_log_exp_and_others":
                fns = set(fns) - {AF.Exp, AF.Ln, AF.Copy, AF.Square}
            outm[name] = fns
        return outm

    _bacc_mod.get_activation_tables = _filtered_tables
    try:
        nc.compile()
    finally:
        _bacc_mod.get_activation_tables = _orig_tables
    return nc


_GRAPH_CACHE = {}


def _get_graph(apply_affine: bool) -> bass.Bass:
    if apply_affine not in _GRAPH_CACHE:
        _GRAPH_CACHE[apply_affine] = _build_graph(apply_affine)
    return _GRAPH_CACHE[apply_affine]


def _prep_in_maps(x, Wq, bq, Wk, bk, Wv, bv, gamma, beta, apply_affine):
    scale = 1.0 / np.sqrt(np.float64(DK))
    in_maps = []
    for i in range(NCORES):
        dsl = slice(DC * i, DC * (i + 1))
        hsl = slice(HPC * i, HPC * (i + 1))
        x_sl = x[:, :, dsl]                       # [B, S, 128]
        xx_np = x_sl.transpose(0, 2, 1)           # [B, 128, S]
        Wq_h = Wq[hsl].astype(np.float64)
        bq_h = bq[hsl].astype(np.float64)
        Wk_h = Wk[hsl].astype(np.float64)
        # A_h = Wk Wq^T * scale ; z = A x_s ; score += (Wk bq * scale) . x_t
        A_h = np.einsum("hde,hfe->hdf", Wk_h, Wq_h) * scale   # [h, dK, dQ]
        u_h = np.einsum("hde,he->hd", Wk_h, bq_h) * scale     # [h, dK]
        wz = np.zeros((DC, DC), np.float64)
        for hh in range(HPC):
            blk = slice(DK * hh, DK * hh + DK)
            wz[blk, blk] = A_h[hh].T
        wv = np.zeros((DC, DK), np.float64)
        for hh in range(HPC):
            wv[DK * hh : DK * hh + DK, :] = Wv[hsl][hh]
        # per-key bias bb[b, hh, t] = u_h . x_h[:, t]; folded into V as e^bb
        bb = np.einsum("hd,bthd->bht", u_h,
                       x_sl.reshape(x.shape[0], x.shape[1], HPC, DK).astype(np.float64))
        ebb_bht = np.exp(bb)                                   # [B, HPC, S]
        ebb_np = ebb_bht.reshape(x.shape[0], HPC, S // 128, 128).transpose(0, 1, 3, 2)
        # xv: x scaled by e^bb per head-half, transposed (V-projection input)
        xv_np = xx_np * ebb_bht.repeat(DK, axis=1)             # [B, 128, S]
        xs_np = x_sl + bv[hsl].reshape(1, 1, DC)
        m = {
            "xx": np.ascontiguousarray(xx_np).astype(BF_NP),
            "xv": np.ascontiguousarray(xv_np).astype(BF_NP),
            "xs": np.ascontiguousarray(xs_np).astype(np.float32),
            "wpack": np.ascontiguousarray(
                np.concatenate([wz, wv], axis=1)
            ).astype(BF_NP),
            "ebb": np.ascontiguousarray(ebb_np).astype(np.float32),
        }
        if apply_affine:
            m["gam"] = np.ascontiguousarray(
                np.tile(gamma[dsl][None, :], (128, 1))
            ).astype(np.float32)
            m["bet"] = np.ascontiguousarray(
                np.tile(beta[dsl][None, :], (128, 1))
            ).astype(np.float32)
        in_maps.append(m)
    return in_maps


def kernel(x, Wq, bq, Wk, bk, Wv, bv, gamma, beta):
    global LAST_RESULTS
    x = np.asarray(x, np.float32)
    Wq = np.asarray(Wq, np.float32)
    bq = np.asarray(bq, np.float32)
    Wk = np.asarray(Wk, np.float32)
    bk = np.asarray(bk, np.float32)
    Wv = np.asarray(Wv, np.float32)
    bv = np.asarray(bv, np.float32)
    gamma = np.asarray(gamma, np.float32)
    beta = np.asarray(beta, np.float32)

    apply_affine = not (
        np.allclose(gamma, 1.0, atol=0.0, rtol=0.0)
        and np.allclose(beta, 0.0, atol=0.0, rtol=0.0)
    )
    fake_ar = bool(int(os.environ.get("KERNEL_FAKE_AR", "0")))
    nc = _get_graph(apply_affine) if not fake_ar else _build_graph(apply_affine, fake_ar=True)

    in_maps = _prep_in_maps(x, Wq, bq, Wk, bk, Wv, bv, gamma, beta, apply_affine)

    res = run_bass_kernel_spmd(
        nc,
        in_maps,
        core_ids=list(range(NCORES)),
        trace=bool(int(os.environ.get("KERNEL_TRACE", "0"))),
    )
    LAST_RESULTS = res
    outs = [np.asarray(r["out"], np.float32) for r in res.results]
    return np.concatenate(outs, axis=2)


if __name__ == "__main__":
    nc = _build_graph(False)
    print("graph built ok:", len(nc.inst_map), "instructions")
